# revision 20
# baseline (speedup 1.0000x reference)
"""Multi-head attention with additive positional bias on 8 Trainium2 cores.

Problem: q,k,v [8, 1024, 512] fp32, pos_bias [1, 8, 1024, 1024] fp32,
8 heads x head_dim 64, out = softmax(q@k^T * scale + bias) @ v.

Sharding: one head per NeuronCore (tensor parallel over heads).

Per-core pipeline (scores transposed: S^T[j,i], j on partitions; q is
pre-scaled by A*SCALE on the host so psum holds A*s where A=128/ln2).
Each engine's queue depends only on the PE, never on another drain engine
(strict per-engine FIFOs make cross-engine chains serialize the pipeline):
  - PE: QK^T bf16 K=64 row-tiled (j-tile pairs co-execute on the two
    64-row PE halves); bias inject via identity-stationary matmul for
    tiles 4,6 (psum += A*b); PV matmul for the previous batch interleaved.
  - ScalarE: true exp (scale=1/A) for tiles 0,2,4,6 + half the po evac.
  - VectorE: Schraudolph bit-trick exp for tiles 1,3,5,7:
    bitcast_bf16(int16(A*s + 16249 + A*b)) via one scalar_tensor_tensor
    against the resident bf16 A*b table; + half the po evac.
  - GpSimd: exp(bias) multiply for tiles 0,2 (consumed only next period,
    so its latency is off the critical path).
  - PV: po[dv,i] += [V|ones]^T @ P^T over j-tiles; 65th row = softmax
    denominators; host divides and untransposes.
"""

import numpy as np
from contextlib import ExitStack

import concourse.bacc as bacc
import concourse.bass as bass
import concourse.mybir as mybir
import concourse.tile as tile
from concourse.bass_utils import run_bass_kernel_spmd

B = 8          # batch
S = 1024       # sequence length
D = 512        # model dim
H = 8          # heads
HD = 64        # head dim
NT = S // 128  # 128-row j-tiles per sequence
SCALE = HD ** -0.5

A_SCH = 128.0 / np.log(2.0)          # Schraudolph scale (bf16: 2^7 mantissa)
DELTA = -7.0                         # minimax-ish centering of the bit trick
SCH_TILES = (1, 3, 5, 6, 7)             # j-tiles drained by the DVE bit-trick exp
GPS_MUL_TILES = (0, 2, 4)               # exp-tiles whose bias-mul runs on GpSimd
INJ_TILES = ()                   # exp-tiles whose bias is PE-injected
ROW_TILED = True                     # K=64 PE row-tiling for QK^T

_PROGRAM = None


def _emit(ctx, tc, out, qt, kt, vp, bb, eb, ident):
    nc = tc.nc
    f32 = mybir.dt.float32
    bf16 = mybir.dt.bfloat16
    i16 = mybir.dt.int16

    singles = ctx.enter_context(tc.tile_pool(name="singles", bufs=1))
    qk_pool = ctx.enter_context(tc.tile_pool(name="qk_pool", bufs=2))
    v_pool = ctx.enter_context(tc.tile_pool(name="v_pool", bufs=2))
    e_pool = ctx.enter_context(tc.tile_pool(name="e_pool", bufs=4))
    p_pool = ctx.enter_context(tc.tile_pool(name="p_pool", bufs=12))
    o_pool = ctx.enter_context(tc.tile_pool(name="o_pool", bufs=2))
    ps_s = ctx.enter_context(tc.tile_pool(name="ps_s", bufs=3, space="PSUM"))
    ps_o = ctx.enter_context(tc.tile_pool(name="ps_o", bufs=1, space="PSUM"))

    # batch-0 inputs first so compute starts immediately; then tables in
    # first-use order (inject-critical bb tiles before the drain tables)
    qtb0 = qk_pool.tile([128, S], bf16, name="qtb0", tag="qtb")
    nc.sync.dma_start(out=qtb0, in_=qt[0])
    ktb0 = qk_pool.tile([128, S], bf16, name="ktb0", tag="ktb")
    nc.sync.dma_start(out=ktb0, in_=kt[0])
    idt = None
    bb_tiles = [None] * NT
    eb_tiles = {}
    # tables in first-use order: tile t's drain table (eb for GPS-mul tiles,
    # bb for Schraudolph tiles) right when batch 0 reaches it
    for t in range(NT):
        if t in GPS_MUL_TILES:
            ebt = singles.tile([128, S], bf16, name=f"ebt{t}")
            nc.gpsimd.dma_start(out=ebt, in_=eb[t * 128:(t + 1) * 128, :])
            eb_tiles[t] = ebt
        if t in SCH_TILES or t in INJ_TILES:
            bbt = singles.tile([128, S], bf16, name=f"bbt{t}")
            nc.gpsimd.dma_start(out=bbt, in_=bb[t * 128:(t + 1) * 128, :])
            bb_tiles[t] = bbt
    vpb0 = v_pool.tile([128, NT, 128], bf16, name="vpb0", tag="vpb")
    nc.sync.dma_start(out=vpb0, in_=vp[0])

    prev = None  # (ptiles, vpb) of previous batch, for interleaved mm2

    def inject(ps, t):
        """psum[j,i] = A*bias[j,i] via identity-stationary matmul."""
        for c in range(2):
            cs = slice(c * 512, (c + 1) * 512)
            nc.tensor.matmul(
                ps[:, cs], idt, bb_tiles[t][:, cs], start=True, stop=False
            )

    def mm1(qtb, ktb, ps, t):
        """psum (+)= A*s for j-tile t."""
        if ROW_TILED:
            r = slice(64, 128) if (t % 2) else slice(0, 64)
        else:
            r = slice(0, 128)
        first = t not in INJ_TILES
        for c in range(2):
            cs = slice(c * 512, (c + 1) * 512)
            nc.tensor.matmul(
                ps[:, cs],
                ktb[r, t * 128:(t + 1) * 128],
                qtb[r, cs],
                start=first,
                stop=True,
            )

    def drain(ps, t):
        """psum -> P tile (bf16 ~ exp(s+b))."""
        pt = p_pool.tile([128, S], bf16, tag="pt")
        if t in SCH_TILES:
            # int16(A*s + (16256-7) + A*b) bitcast to bf16 = ~exp(s+b)
            nc.vector.scalar_tensor_tensor(
                pt.bitcast(i16),
                ps,
                float(127.0 * 128.0 + DELTA),
                bb_tiles[t],
                mybir.AluOpType.add,
                mybir.AluOpType.add,
            )
        elif t in INJ_TILES:
            # bias already in psum
            nc.scalar.activation(
                pt, ps, mybir.ActivationFunctionType.Exp, scale=float(1.0 / A_SCH)
            )
        else:
            et = e_pool.tile([128, S], bf16, tag="et")
            nc.scalar.activation(
                et, ps, mybir.ActivationFunctionType.Exp, scale=float(1.0 / A_SCH)
            )
            nc.gpsimd.tensor_mul(pt, et, eb_tiles[t])
        return pt

    def mm2(po, ptiles, vpb, t):
        for c in range(2):
            cs = slice(c * 512, (c + 1) * 512)
            nc.tensor.matmul(
                po[:, cs],
                vpb[:, t, :],
                ptiles[t][:, cs],
                start=(t == 0),
                stop=(t == NT - 1),
            )

    def finish(po_prev):
        osb = o_pool.tile([HD + 1, S], f32, tag="osb")
        # split the evacuation across ScalarE and VectorE
        nc.scalar.activation(
            osb, po_prev[0][0:HD + 1, :], mybir.ActivationFunctionType.Copy
        )
        nc.scalar.dma_start(out=out[po_prev[1]], in_=osb)

    for b in range(B):
        if b == 0:
            qtb, ktb, vpb = qtb0, ktb0, vpb0
        else:
            qtb = qk_pool.tile([128, S], bf16, tag="qtb")
            nc.sync.dma_start(out=qtb, in_=qt[b])
            ktb = qk_pool.tile([128, S], bf16, tag="ktb")
            nc.sync.dma_start(out=ktb, in_=kt[b])
            vpb = v_pool.tile([128, NT, 128], bf16, tag="vpb")
            nc.sync.dma_start(out=vpb, in_=vp[b])

        po = None
        if prev is not None:
            po = ps_o.tile([128, S], f32, tag="po")
        ptiles = []
        for p in range(NT // 2):
            t0, t1 = 2 * p, 2 * p + 1
            ps0 = ps_s.tile([128, S], f32, tag="ps")
            ps1 = ps_s.tile([128, S], f32, tag="ps")
            if t0 in INJ_TILES:
                inject(ps0, t0)
            if t1 in INJ_TILES:
                inject(ps1, t1)
            # pair emitted back-to-back on PE so the K=64 halves co-execute
            mm1(qtb, ktb, ps0, t0)
            mm1(qtb, ktb, ps1, t1)
            if prev is not None:
                mm2(po, prev[0], prev[1], t0)
                mm2(po, prev[0], prev[1], t1)
            ptiles.append(drain(ps0, t0))
            ptiles.append(drain(ps1, t1))
        if prev is not None:
            finish((po, b - 1))
        prev = (ptiles, vpb)

    po = ps_o.tile([128, S], f32, tag="po")
    for t in range(NT):
        mm2(po, prev[0], prev[1], t)
    finish((po, B - 1))


def _build_program():
    nc = bacc.Bacc("TRN2", target_bir_lowering=False, debug=False)
    bf16 = mybir.dt.bfloat16
    qt = nc.dram_tensor("qt", [B, 128, S], bf16, kind="ExternalInput").ap()
    kt = nc.dram_tensor("kt", [B, 128, S], bf16, kind="ExternalInput").ap()
    vp = nc.dram_tensor("vp", [B, 128, NT, 128], bf16, kind="ExternalInput").ap()
    bb = nc.dram_tensor("bb", [S, S], bf16, kind="ExternalInput").ap()
    eb = nc.dram_tensor("eb", [S, S], bf16, kind="ExternalInput").ap()
    ident = nc.dram_tensor("ident", [128, 128], bf16, kind="ExternalInput").ap()
    out = nc.dram_tensor(
        "out", [B, HD + 1, S], mybir.dt.float32, kind="ExternalOutput"
    ).ap()
    with tile.TileContext(nc) as tc, ExitStack() as ctx:
        _emit(ctx, tc, out, qt, kt, vp, bb, eb, ident)
    nc.compile()
    return nc


def get_program():
    global _PROGRAM
    if _PROGRAM is None:
        _PROGRAM = _build_program()
    return _PROGRAM


def make_in_maps(q, k, v, pos_bias):
    import ml_dtypes

    nbf16 = ml_dtypes.bfloat16
    q4 = q.reshape(B, S, H, HD)
    k4 = k.reshape(B, S, H, HD)
    v4 = v.reshape(B, S, H, HD)
    ones = np.ones((B, S, 1), np.float32)
    qscale = np.float32(SCALE * A_SCH)
    ident = np.eye(128, dtype=nbf16)
    in_maps = []
    for h in range(H):
        qt = np.empty((B, 128, S), nbf16)
        qt[:, :HD, :] = (q4[:, :, h, :].transpose(0, 2, 1) * qscale).astype(nbf16)
        qt[:, HD:, :] = qt[:, :HD, :]
        kt = np.empty((B, 128, S), nbf16)
        kt[:, :HD, :] = k4[:, :, h, :].transpose(0, 2, 1).astype(nbf16)
        kt[:, HD:, :] = kt[:, :HD, :]
        vp = np.concatenate(
            [v4[:, :, h, :], ones, np.zeros((B, S, 63), np.float32)], axis=2
        )  # [B, S, 128]
        vp = np.ascontiguousarray(
            vp.reshape(B, NT, 128, 128).transpose(0, 2, 1, 3)
        ).astype(nbf16)  # [B, 128, NT, 128]
        btT = np.ascontiguousarray(pos_bias[0, h].T).astype(np.float32)  # [j, i]
        bb = (A_SCH * btT).astype(nbf16)
        eb = np.exp(btT).astype(nbf16)
        in_maps.append(
            {"qt": qt, "kt": kt, "vp": vp, "bb": bb, "eb": eb, "ident": ident}
        )
    return in_maps


def assemble_output(results):
    out = np.empty((B, S, D), np.float32)
    for h in range(H):
        o = results[h]["out"]  # [B, 65, S]
        normed = o[:, :HD, :] / o[:, HD:HD + 1, :]
        out[:, :, h * HD:(h + 1) * HD] = normed.transpose(0, 2, 1)
    return out


def kernel(q, k, v, pos_bias):
    nc = get_program()
    in_maps = make_in_maps(
        np.asarray(q, np.float32),
        np.asarray(k, np.float32),
        np.asarray(v, np.float32),
        np.asarray(pos_bias, np.float32),
    )
    res = run_bass_kernel_spmd(nc, in_maps, list(range(H))).results
    return assemble_output(res)


# revision 21
# speedup vs baseline: 1.1966x; 1.1966x over previous
"""Multi-head attention with additive positional bias on 8 Trainium2 cores.

Problem: q,k,v [8, 1024, 512] fp32, pos_bias [1, 8, 1024, 1024] fp32,
8 heads x head_dim 64, out = softmax(q@k^T * scale + bias) @ v.

Sharding: one head per NeuronCore (tensor parallel over heads).

Per-core pipeline (scores transposed: S^T[j,i], j on partitions; q is
pre-scaled by A*SCALE on the host so psum holds A*s where A=128/ln2).
Each engine's queue depends only on the PE, never on another drain engine
(strict per-engine FIFOs make cross-engine chains serialize the pipeline):
  - PE: QK^T bf16 K=64 row-tiled (j-tile pairs co-execute on the two
    64-row PE halves); bias inject via identity-stationary matmul for
    tiles 4,6 (psum += A*b); PV matmul for the previous batch interleaved.
  - ScalarE: true exp (scale=1/A) for tiles 0,2,4,6 + half the po evac.
  - VectorE: Schraudolph bit-trick exp for tiles 1,3,5,7:
    bitcast_bf16(int16(A*s + 16249 + A*b)) via one scalar_tensor_tensor
    against the resident bf16 A*b table; + half the po evac.
  - GpSimd: exp(bias) multiply for tiles 0,2 (consumed only next period,
    so its latency is off the critical path).
  - PV: po[dv,i] += [V|ones]^T @ P^T over j-tiles; 65th row = softmax
    denominators; host divides and untransposes.
"""

import numpy as np
from contextlib import ExitStack

import concourse.bacc as bacc
import concourse.bass as bass
import concourse.mybir as mybir
import concourse.tile as tile
from concourse.bass_utils import run_bass_kernel_spmd

B = 8          # batch
S = 1024       # sequence length
D = 512        # model dim
H = 8          # heads
HD = 64        # head dim
NT = S // 128  # 128-row j-tiles per sequence
SCALE = HD ** -0.5

A_SCH = 128.0 / np.log(2.0)          # Schraudolph scale (bf16: 2^7 mantissa)
DELTA = -7.0                         # minimax-ish centering of the bit trick
SCH_TILES = (1, 3, 5, 6, 7)             # j-tiles drained by the DVE bit-trick exp
GPS_MUL_TILES = (0, 2, 4)               # exp-tiles whose bias-mul runs on GpSimd
INJ_TILES = ()                   # exp-tiles whose bias is PE-injected
ROW_TILED = True                     # K=64 PE row-tiling for QK^T

_PROGRAM = None


def _emit(ctx, tc, out, qt, kt, vp, bb, eb, ident):
    nc = tc.nc
    f32 = mybir.dt.float32
    bf16 = mybir.dt.bfloat16
    i16 = mybir.dt.int16

    singles = ctx.enter_context(tc.tile_pool(name="singles", bufs=1))
    qk_pool = ctx.enter_context(tc.tile_pool(name="qk_pool", bufs=2))
    v_pool = ctx.enter_context(tc.tile_pool(name="v_pool", bufs=2))
    e_pool = ctx.enter_context(tc.tile_pool(name="e_pool", bufs=4))
    p_pool = ctx.enter_context(tc.tile_pool(name="p_pool", bufs=12))
    o_pool = ctx.enter_context(tc.tile_pool(name="o_pool", bufs=2))
    ps_s = ctx.enter_context(tc.tile_pool(name="ps_s", bufs=3, space="PSUM"))
    ps_o = ctx.enter_context(tc.tile_pool(name="ps_o", bufs=1, space="PSUM"))

    # batch-0 inputs first so compute starts immediately; then tables in
    # first-use order (inject-critical bb tiles before the drain tables)
    qtb0 = qk_pool.tile([128, S], bf16, name="qtb0", tag="qtb")
    nc.sync.dma_start(out=qtb0, in_=qt[0])
    ktb0 = qk_pool.tile([128, S], bf16, name="ktb0", tag="ktb")
    nc.sync.dma_start(out=ktb0, in_=kt[0])
    idt = None
    bb_tiles = [None] * NT
    eb_tiles = {}
    # tables in first-use order: tile t's drain table (eb for GPS-mul tiles,
    # bb for Schraudolph tiles) right when batch 0 reaches it
    for t in range(NT):
        if t in GPS_MUL_TILES:
            ebt = singles.tile([128, S], bf16, name=f"ebt{t}")
            nc.sync.dma_start(out=ebt, in_=eb[t * 128:(t + 1) * 128, :])
            eb_tiles[t] = ebt
        if t in SCH_TILES or t in INJ_TILES:
            bbt = singles.tile([128, S], bf16, name=f"bbt{t}")
            nc.sync.dma_start(out=bbt, in_=bb[t * 128:(t + 1) * 128, :])
            bb_tiles[t] = bbt
    vpb0 = v_pool.tile([128, NT, 128], bf16, name="vpb0", tag="vpb")
    nc.sync.dma_start(out=vpb0, in_=vp[0])

    prev = None  # (ptiles, vpb) of previous batch, for interleaved mm2

    def inject(ps, t):
        """psum[j,i] = A*bias[j,i] via identity-stationary matmul."""
        for c in range(2):
            cs = slice(c * 512, (c + 1) * 512)
            nc.tensor.matmul(
                ps[:, cs], idt, bb_tiles[t][:, cs], start=True, stop=False
            )

    def mm1(qtb, ktb, ps, t):
        """psum (+)= A*s for j-tile t."""
        if ROW_TILED:
            r = slice(64, 128) if (t % 2) else slice(0, 64)
        else:
            r = slice(0, 128)
        first = t not in INJ_TILES
        for c in range(2):
            cs = slice(c * 512, (c + 1) * 512)
            nc.tensor.matmul(
                ps[:, cs],
                ktb[r, t * 128:(t + 1) * 128],
                qtb[r, cs],
                start=first,
                stop=True,
            )

    def drain(ps, t):
        """psum -> P tile (bf16 ~ exp(s+b))."""
        pt = p_pool.tile([128, S], bf16, tag="pt")
        if t in SCH_TILES:
            # int16(A*s + (16256-7) + A*b) bitcast to bf16 = ~exp(s+b)
            nc.vector.scalar_tensor_tensor(
                pt.bitcast(i16),
                ps,
                float(127.0 * 128.0 + DELTA),
                bb_tiles[t],
                mybir.AluOpType.add,
                mybir.AluOpType.add,
            )
        elif t in INJ_TILES:
            # bias already in psum
            nc.scalar.activation(
                pt, ps, mybir.ActivationFunctionType.Exp, scale=float(1.0 / A_SCH)
            )
        else:
            et = e_pool.tile([128, S], bf16, tag="et")
            nc.scalar.activation(
                et, ps, mybir.ActivationFunctionType.Exp, scale=float(1.0 / A_SCH)
            )
            nc.gpsimd.tensor_mul(pt, et, eb_tiles[t])
        return pt

    def mm2(po, ptiles, vpb, t):
        for c in range(2):
            cs = slice(c * 512, (c + 1) * 512)
            nc.tensor.matmul(
                po[:, cs],
                vpb[:, t, :],
                ptiles[t][:, cs],
                start=(t == 0),
                stop=(t == NT - 1),
            )

    def finish(po_prev):
        osb = o_pool.tile([HD + 1, S], f32, tag="osb")
        # split the evacuation across ScalarE and VectorE
        nc.scalar.activation(
            osb, po_prev[0][0:HD + 1, :], mybir.ActivationFunctionType.Copy
        )
        nc.sync.dma_start(out=out[po_prev[1]], in_=osb)

    for b in range(B):
        if b == 0:
            qtb, ktb, vpb = qtb0, ktb0, vpb0
        else:
            qtb = qk_pool.tile([128, S], bf16, tag="qtb")
            nc.sync.dma_start(out=qtb, in_=qt[b])
            ktb = qk_pool.tile([128, S], bf16, tag="ktb")
            nc.sync.dma_start(out=ktb, in_=kt[b])
            vpb = v_pool.tile([128, NT, 128], bf16, tag="vpb")
            nc.sync.dma_start(out=vpb, in_=vp[b])

        po = None
        if prev is not None:
            po = ps_o.tile([128, S], f32, tag="po")
        ptiles = []
        for p in range(NT // 2):
            t0, t1 = 2 * p, 2 * p + 1
            ps0 = ps_s.tile([128, S], f32, tag="ps")
            ps1 = ps_s.tile([128, S], f32, tag="ps")
            if t0 in INJ_TILES:
                inject(ps0, t0)
            if t1 in INJ_TILES:
                inject(ps1, t1)
            # pair emitted back-to-back on PE so the K=64 halves co-execute
            mm1(qtb, ktb, ps0, t0)
            mm1(qtb, ktb, ps1, t1)
            if prev is not None:
                mm2(po, prev[0], prev[1], t0)
                mm2(po, prev[0], prev[1], t1)
            ptiles.append(drain(ps0, t0))
            ptiles.append(drain(ps1, t1))
        if prev is not None:
            finish((po, b - 1))
        prev = (ptiles, vpb)

    po = ps_o.tile([128, S], f32, tag="po")
    for t in range(NT):
        mm2(po, prev[0], prev[1], t)
    finish((po, B - 1))


def _build_program():
    nc = bacc.Bacc("TRN2", target_bir_lowering=False, debug=False)
    bf16 = mybir.dt.bfloat16
    qt = nc.dram_tensor("qt", [B, 128, S], bf16, kind="ExternalInput").ap()
    kt = nc.dram_tensor("kt", [B, 128, S], bf16, kind="ExternalInput").ap()
    vp = nc.dram_tensor("vp", [B, 128, NT, 128], bf16, kind="ExternalInput").ap()
    bb = nc.dram_tensor("bb", [S, S], bf16, kind="ExternalInput").ap()
    eb = nc.dram_tensor("eb", [S, S], bf16, kind="ExternalInput").ap()
    ident = nc.dram_tensor("ident", [128, 128], bf16, kind="ExternalInput").ap()
    out = nc.dram_tensor(
        "out", [B, HD + 1, S], mybir.dt.float32, kind="ExternalOutput"
    ).ap()
    with tile.TileContext(nc) as tc, ExitStack() as ctx:
        _emit(ctx, tc, out, qt, kt, vp, bb, eb, ident)
    nc.compile()
    return nc


def get_program():
    global _PROGRAM
    if _PROGRAM is None:
        _PROGRAM = _build_program()
    return _PROGRAM


def make_in_maps(q, k, v, pos_bias):
    import ml_dtypes

    nbf16 = ml_dtypes.bfloat16
    q4 = q.reshape(B, S, H, HD)
    k4 = k.reshape(B, S, H, HD)
    v4 = v.reshape(B, S, H, HD)
    ones = np.ones((B, S, 1), np.float32)
    qscale = np.float32(SCALE * A_SCH)
    ident = np.eye(128, dtype=nbf16)
    in_maps = []
    for h in range(H):
        qt = np.empty((B, 128, S), nbf16)
        qt[:, :HD, :] = (q4[:, :, h, :].transpose(0, 2, 1) * qscale).astype(nbf16)
        qt[:, HD:, :] = qt[:, :HD, :]
        kt = np.empty((B, 128, S), nbf16)
        kt[:, :HD, :] = k4[:, :, h, :].transpose(0, 2, 1).astype(nbf16)
        kt[:, HD:, :] = kt[:, :HD, :]
        vp = np.concatenate(
            [v4[:, :, h, :], ones, np.zeros((B, S, 63), np.float32)], axis=2
        )  # [B, S, 128]
        vp = np.ascontiguousarray(
            vp.reshape(B, NT, 128, 128).transpose(0, 2, 1, 3)
        ).astype(nbf16)  # [B, 128, NT, 128]
        btT = np.ascontiguousarray(pos_bias[0, h].T).astype(np.float32)  # [j, i]
        bb = (A_SCH * btT).astype(nbf16)
        eb = np.exp(btT).astype(nbf16)
        in_maps.append(
            {"qt": qt, "kt": kt, "vp": vp, "bb": bb, "eb": eb, "ident": ident}
        )
    return in_maps


def assemble_output(results):
    out = np.empty((B, S, D), np.float32)
    for h in range(H):
        o = results[h]["out"]  # [B, 65, S]
        normed = o[:, :HD, :] / o[:, HD:HD + 1, :]
        out[:, :, h * HD:(h + 1) * HD] = normed.transpose(0, 2, 1)
    return out


def kernel(q, k, v, pos_bias):
    nc = get_program()
    in_maps = make_in_maps(
        np.asarray(q, np.float32),
        np.asarray(k, np.float32),
        np.asarray(v, np.float32),
        np.asarray(pos_bias, np.float32),
    )
    res = run_bass_kernel_spmd(nc, in_maps, list(range(H))).results
    return assemble_output(res)


# revision 22
# speedup vs baseline: 1.2018x; 1.0044x over previous
"""Multi-head attention with additive positional bias on 8 Trainium2 cores.

Problem: q,k,v [8, 1024, 512] fp32, pos_bias [1, 8, 1024, 1024] fp32,
8 heads x head_dim 64, out = softmax(q@k^T * scale + bias) @ v.

Sharding: one head per NeuronCore (tensor parallel over heads).

Per-core pipeline (scores transposed: S^T[j,i], j on partitions; q is
pre-scaled by A*SCALE on the host so psum holds A*s where A=128/ln2).
The softmax exp is the hard bottleneck (1M psum values per batch must
leave PSUM through ScalarE/VectorE, ~130/110 G elem/s), so the drain is
split so each engine's queue depends only on the PE, never on another
drain engine (strict per-engine FIFOs serialize cross-engine chains):
  - PE: QK^T bf16 (16x N=512 matmuls, 1 col/cycle) + PV matmul for the
    previous batch interleaved between the QK pairs.
  - VectorE: Schraudolph bit-trick exp for j-tiles 1,3,5,6,7:
    bitcast_bf16(int16(A*s + 16249 + A*b)) via one scalar_tensor_tensor
    against a resident bf16 A*b table (int16 out aliases the bf16 tile).
  - ScalarE: true exp (free affine scale=1/A) for tiles 0,2,4 + po evac.
  - GpSimd: exp(bias) multiply for tiles 0,2,4 (their P tiles are only
    consumed by next period's PV, so GpSimd latency is off-critical-path).
  - PV: po[dv,i] += [V|ones]^T @ P^T over j-tiles; 65th row = softmax
    denominators via the ones column; host divides and untransposes.
Table DMAs are emitted in first-use order behind batch 0's q/k so the
prologue overlaps compute (the single sync DMA ring is serial).
"""

import numpy as np
from contextlib import ExitStack

import concourse.bacc as bacc
import concourse.bass as bass
import concourse.mybir as mybir
import concourse.tile as tile
from concourse.bass_utils import run_bass_kernel_spmd

B = 8          # batch
S = 1024       # sequence length
D = 512        # model dim
H = 8          # heads
HD = 64        # head dim
NT = S // 128  # 128-row j-tiles per sequence
SCALE = HD ** -0.5

A_SCH = 128.0 / np.log(2.0)          # Schraudolph scale (bf16: 2^7 mantissa)
DELTA = -7.0                         # minimax-ish centering of the bit trick
SCH_TILES = (1, 3, 5, 6, 7)             # j-tiles drained by the DVE bit-trick exp
GPS_MUL_TILES = (0, 2, 4)               # exp-tiles whose bias-mul runs on GpSimd
INJ_TILES = ()                   # exp-tiles whose bias is PE-injected
ROW_TILED = True                     # K=64 PE row-tiling for QK^T

_PROGRAM = None


def _emit(ctx, tc, out, qt, kt, vp, bb, eb, ident):
    nc = tc.nc
    f32 = mybir.dt.float32
    bf16 = mybir.dt.bfloat16
    i16 = mybir.dt.int16

    singles = ctx.enter_context(tc.tile_pool(name="singles", bufs=1))
    qk_pool = ctx.enter_context(tc.tile_pool(name="qk_pool", bufs=2))
    v_pool = ctx.enter_context(tc.tile_pool(name="v_pool", bufs=2))
    e_pool = ctx.enter_context(tc.tile_pool(name="e_pool", bufs=4))
    p_pool = ctx.enter_context(tc.tile_pool(name="p_pool", bufs=12))
    o_pool = ctx.enter_context(tc.tile_pool(name="o_pool", bufs=2))
    ps_s = ctx.enter_context(tc.tile_pool(name="ps_s", bufs=3, space="PSUM"))
    ps_o = ctx.enter_context(tc.tile_pool(name="ps_o", bufs=1, space="PSUM"))

    # batch-0 inputs first so compute starts immediately; then tables in
    # first-use order (inject-critical bb tiles before the drain tables)
    qtb0 = qk_pool.tile([128, S], bf16, name="qtb0", tag="qtb")
    nc.sync.dma_start(out=qtb0, in_=qt[0])
    ktb0 = qk_pool.tile([128, S], bf16, name="ktb0", tag="ktb")
    nc.sync.dma_start(out=ktb0, in_=kt[0])
    idt = None
    bb_tiles = [None] * NT
    eb_tiles = {}
    # tables in first-use order: tile t's drain table (eb for GPS-mul tiles,
    # bb for Schraudolph tiles) right when batch 0 reaches it
    for t in range(NT):
        if t in GPS_MUL_TILES:
            ebt = singles.tile([128, S], bf16, name=f"ebt{t}")
            nc.sync.dma_start(out=ebt, in_=eb[t * 128:(t + 1) * 128, :])
            eb_tiles[t] = ebt
        if t in SCH_TILES or t in INJ_TILES:
            bbt = singles.tile([128, S], bf16, name=f"bbt{t}")
            nc.sync.dma_start(out=bbt, in_=bb[t * 128:(t + 1) * 128, :])
            bb_tiles[t] = bbt
    vpb0 = v_pool.tile([128, NT, 128], bf16, name="vpb0", tag="vpb")
    nc.sync.dma_start(out=vpb0, in_=vp[0])

    prev = None  # (ptiles, vpb) of previous batch, for interleaved mm2

    def inject(ps, t):
        """psum[j,i] = A*bias[j,i] via identity-stationary matmul."""
        for c in range(2):
            cs = slice(c * 512, (c + 1) * 512)
            nc.tensor.matmul(
                ps[:, cs], idt, bb_tiles[t][:, cs], start=True, stop=False
            )

    def mm1(qtb, ktb, ps, t):
        """psum (+)= A*s for j-tile t."""
        if ROW_TILED:
            r = slice(64, 128) if (t % 2) else slice(0, 64)
        else:
            r = slice(0, 128)
        first = t not in INJ_TILES
        for c in range(2):
            cs = slice(c * 512, (c + 1) * 512)
            nc.tensor.matmul(
                ps[:, cs],
                ktb[r, t * 128:(t + 1) * 128],
                qtb[r, cs],
                start=first,
                stop=True,
            )

    def drain(ps, t):
        """psum -> P tile (bf16 ~ exp(s+b))."""
        pt = p_pool.tile([128, S], bf16, tag="pt")
        if t in SCH_TILES:
            # int16(A*s + (16256-7) + A*b) bitcast to bf16 = ~exp(s+b)
            nc.vector.scalar_tensor_tensor(
                pt.bitcast(i16),
                ps,
                float(127.0 * 128.0 + DELTA),
                bb_tiles[t],
                mybir.AluOpType.add,
                mybir.AluOpType.add,
            )
        elif t in INJ_TILES:
            # bias already in psum
            nc.scalar.activation(
                pt, ps, mybir.ActivationFunctionType.Exp, scale=float(1.0 / A_SCH)
            )
        else:
            et = e_pool.tile([128, S], bf16, tag="et")
            nc.scalar.activation(
                et, ps, mybir.ActivationFunctionType.Exp, scale=float(1.0 / A_SCH)
            )
            nc.gpsimd.tensor_mul(pt, et, eb_tiles[t])
        return pt

    def mm2(po, ptiles, vpb, t):
        for c in range(2):
            cs = slice(c * 512, (c + 1) * 512)
            nc.tensor.matmul(
                po[:, cs],
                vpb[:, t, :],
                ptiles[t][:, cs],
                start=(t == 0),
                stop=(t == NT - 1),
            )

    def finish(po_prev):
        osb = o_pool.tile([HD + 1, S], f32, tag="osb")
        # split the evacuation across ScalarE and VectorE
        nc.scalar.activation(
            osb, po_prev[0][0:HD + 1, :], mybir.ActivationFunctionType.Copy
        )
        nc.sync.dma_start(out=out[po_prev[1]], in_=osb)

    for b in range(B):
        if b == 0:
            qtb, ktb, vpb = qtb0, ktb0, vpb0
        else:
            qtb = qk_pool.tile([128, S], bf16, tag="qtb")
            nc.sync.dma_start(out=qtb, in_=qt[b])
            ktb = qk_pool.tile([128, S], bf16, tag="ktb")
            nc.sync.dma_start(out=ktb, in_=kt[b])
            vpb = v_pool.tile([128, NT, 128], bf16, tag="vpb")
            nc.sync.dma_start(out=vpb, in_=vp[b])

        po = None
        if prev is not None:
            po = ps_o.tile([128, S], f32, tag="po")
        ptiles = []
        for p in range(NT // 2):
            t0, t1 = 2 * p, 2 * p + 1
            ps0 = ps_s.tile([128, S], f32, tag="ps")
            ps1 = ps_s.tile([128, S], f32, tag="ps")
            if t0 in INJ_TILES:
                inject(ps0, t0)
            if t1 in INJ_TILES:
                inject(ps1, t1)
            # pair emitted back-to-back on PE so the K=64 halves co-execute
            mm1(qtb, ktb, ps0, t0)
            mm1(qtb, ktb, ps1, t1)
            if prev is not None:
                mm2(po, prev[0], prev[1], t0)
                mm2(po, prev[0], prev[1], t1)
            ptiles.append(drain(ps0, t0))
            ptiles.append(drain(ps1, t1))
        if prev is not None:
            finish((po, b - 1))
        prev = (ptiles, vpb)

    po = ps_o.tile([128, S], f32, tag="po")
    for t in range(NT):
        mm2(po, prev[0], prev[1], t)
    finish((po, B - 1))


def _build_program():
    nc = bacc.Bacc("TRN2", target_bir_lowering=False, debug=False)
    bf16 = mybir.dt.bfloat16
    qt = nc.dram_tensor("qt", [B, 128, S], bf16, kind="ExternalInput").ap()
    kt = nc.dram_tensor("kt", [B, 128, S], bf16, kind="ExternalInput").ap()
    vp = nc.dram_tensor("vp", [B, 128, NT, 128], bf16, kind="ExternalInput").ap()
    bb = nc.dram_tensor("bb", [S, S], bf16, kind="ExternalInput").ap()
    eb = nc.dram_tensor("eb", [S, S], bf16, kind="ExternalInput").ap()
    ident = nc.dram_tensor("ident", [128, 128], bf16, kind="ExternalInput").ap()
    out = nc.dram_tensor(
        "out", [B, HD + 1, S], mybir.dt.float32, kind="ExternalOutput"
    ).ap()
    with tile.TileContext(nc) as tc, ExitStack() as ctx:
        _emit(ctx, tc, out, qt, kt, vp, bb, eb, ident)
    nc.compile()
    return nc


def get_program():
    global _PROGRAM
    if _PROGRAM is None:
        _PROGRAM = _build_program()
    return _PROGRAM


def make_in_maps(q, k, v, pos_bias):
    import ml_dtypes

    nbf16 = ml_dtypes.bfloat16
    q4 = q.reshape(B, S, H, HD)
    k4 = k.reshape(B, S, H, HD)
    v4 = v.reshape(B, S, H, HD)
    ones = np.ones((B, S, 1), np.float32)
    qscale = np.float32(SCALE * A_SCH)
    ident = np.eye(128, dtype=nbf16)
    in_maps = []
    for h in range(H):
        qt = np.empty((B, 128, S), nbf16)
        qt[:, :HD, :] = (q4[:, :, h, :].transpose(0, 2, 1) * qscale).astype(nbf16)
        qt[:, HD:, :] = qt[:, :HD, :]
        kt = np.empty((B, 128, S), nbf16)
        kt[:, :HD, :] = k4[:, :, h, :].transpose(0, 2, 1).astype(nbf16)
        kt[:, HD:, :] = kt[:, :HD, :]
        vp = np.concatenate(
            [v4[:, :, h, :], ones, np.zeros((B, S, 63), np.float32)], axis=2
        )  # [B, S, 128]
        vp = np.ascontiguousarray(
            vp.reshape(B, NT, 128, 128).transpose(0, 2, 1, 3)
        ).astype(nbf16)  # [B, 128, NT, 128]
        btT = np.ascontiguousarray(pos_bias[0, h].T).astype(np.float32)  # [j, i]
        bb = (A_SCH * btT).astype(nbf16)
        eb = np.exp(btT).astype(nbf16)
        in_maps.append(
            {"qt": qt, "kt": kt, "vp": vp, "bb": bb, "eb": eb, "ident": ident}
        )
    return in_maps


def assemble_output(results):
    out = np.empty((B, S, D), np.float32)
    for h in range(H):
        o = results[h]["out"]  # [B, 65, S]
        normed = o[:, :HD, :] / o[:, HD:HD + 1, :]
        out[:, :, h * HD:(h + 1) * HD] = normed.transpose(0, 2, 1)
    return out


def kernel(q, k, v, pos_bias):
    nc = get_program()
    in_maps = make_in_maps(
        np.asarray(q, np.float32),
        np.asarray(k, np.float32),
        np.asarray(v, np.float32),
        np.asarray(pos_bias, np.float32),
    )
    res = run_bass_kernel_spmd(nc, in_maps, list(range(H))).results
    return assemble_output(res)
